# revision 23
# baseline (speedup 1.0000x reference)
"""CharCNN embedding kernel for Trainium2 (8 NeuronCores, Bass/Tile).

Computes out[b,t,f] = sum_k conv_w[f, token_ids[b, t+k-pad], k] with zero
padding outside [0,T) — i.e. one_hot(token_ids) -> Conv1d(V->F, k=3, pad=1).

Strategy: data-parallel over batch (B=8 rows, one per core), weight table
replicated and int8-quantized (w_i8 = round(w/delta), global delta =
max|w|/127; the 3-tap int16 accumulation is exact and the host applies
delta, ~7e-3 rel err vs the 2e-2 gate). int8 halves gather bytes so the
DMA window shrinks; the remaining wall is the DVE accumulation (int8-input
adds run ~1/2 the fp16 rate) plus a fixed ~16.5us gpsimd custom-ucode
library load at kernel start.

Key device-side structure per core:
  - fused table TAB [V+1, 3F] int8, TAB[v] = [A|B|C] = conv_w[:, v, :].T
    flattened (A=tap0, B=tap1, C=tap2), zero row at V for edge padding.
  - strip layout: partition p owns positions t = p*NT + j (j = 0..NT-1), so
    the +-1 tap shifts are free-dim shifts inside a partition.
  - per round of G strip-positions one dma_gather of 128*S fused 1.5KB rows
    (dst[i%128, i//128] = TAB[idx[i]], idx streamed in gather wrap order).
    Gathers are issued on SWDGE queues 1-3: nonzero queues hand descriptor
    generation to background Q7 workers (the gpsimd instruction retires in
    ~80ns), so all rounds' descgen runs concurrently and off the critical
    path (on queue 0 it would serialize at ~8.5ns/row). Rounds stay at
    <=1024 descriptors -- larger single calls overflow the ring and wedge
    the device.
  - the idx tile is loaded in two slices (round 0's slice first) so the
    first gather's semaphore wait clears as early as possible.
  - DVE accumulates O = A_shift + B (int8+int8->int16) then O += C_shift
    (int16+int8) into per-round int16 O tiles; every round has its own
    exact-size R/O buffer (unique pool tags) so there are no tile-reuse
    anti-deps and the DVE chain never stalls.
  - strip-edge boundary rows are folded into the first/last round gathers
    as an extra leading/trailing slot; the last round is small to shorten
    the post-DVE tail (store + end barrier).
Output DRAM layout [P, NT, F] int16 reshapes to [T, F] on host (x delta).
"""

from contextlib import ExitStack

import numpy as np

import concourse.bacc as bacc
import concourse.bass as bass
import concourse.mybir as mybir
import concourse.tile as tile
from concourse._compat import with_exitstack
from concourse.bass_utils import run_bass_kernel_spmd

B = 8
T = 4096
F = 512
V = 32000
VP = V + 1  # +1 zero row
K = 3
P = 128
NT = T // P  # 32 positions per partition strip
G_LIST = (1, 7, 8, 8, 7, 1)
NR = len(G_LIST)
G_OFF = tuple(int(x) for x in np.cumsum((0,) + G_LIST))  # round start offsets
# gathered slots per round: data rows + leading bnd slot (r=0) + trailing (last)
S_LIST = tuple(
    G + (1 if r == 0 else 0) + (1 if r == NR - 1 else 0)
    for r, G in enumerate(G_LIST)
)
SMAX = max(S_LIST)
GMAX = max(G_LIST)
SLOT_OFF = tuple(int(x) for x in np.cumsum((0,) + tuple(8 * s for s in S_LIST)))
SW_TOT = SLOT_OFF[-1]  # total idx slots per partition
N_CORES = 8
DMA_SCRATCH = 24576
DT = mybir.dt.int8
DTO = mybir.dt.int16

_nc_cache = {}


@with_exitstack
def _gather_kernel(ctx: ExitStack, tc: tile.TileContext, out_d, tab_d, idxs_d):
    nc = tc.nc

    idxp = ctx.enter_context(tc.tile_pool(name="idx", bufs=1))
    # one exact-size buffer per round (unique tags, bufs=1): no tile-reuse
    # anti-deps anywhere, so the descgen chain never stalls on DVE/stores
    rp = ctx.enter_context(tc.tile_pool(name="rp", bufs=1))
    op = ctx.enter_context(tc.tile_pool(name="op", bufs=1))

    # split idx load: round 0's small slice lands (and its sem fires) sooner,
    # so the first gather issues earlier; the rest follows in parallel
    idxs_t = idxp.tile([P, SW_TOT], mybir.dt.int16)
    nc.sync.dma_start(idxs_t[:, 0 : SLOT_OFF[1]], idxs_d[:, 0 : SLOT_OFF[1]])
    nc.sync.dma_start(idxs_t[:, SLOT_OFF[1] :], idxs_d[:, SLOT_OFF[1] :])

    R = [None] * NR
    O = [None] * NR
    BASE = tuple(1 if r == 0 else 0 for r in range(NR))

    def _finish(r):
        # C boundary at last data row of round r: first data row of round r+1
        G = G_LIST[r]
        nxt = R[r + 1][:, 0:1, 2 * F : 3 * F]
        nc.vector.tensor_add(O[r][:, G - 1 : G, :], O[r][:, G - 1 : G, :], nxt)
        nc.sync.dma_start(out_d[:, G_OFF[r] : G_OFF[r + 1], :], O[r][:, 0:G, :])

    for r, G in enumerate(G_LIST):
        S, b = S_LIST[r], BASE[r]
        Rt = rp.tile([P, S, 3 * F], DT, tag=f"R{r}", name=f"R{r}")
        Ot = op.tile([P, G, F], DTO, tag=f"O{r}", name=f"O{r}")
        R[r] = Rt
        O[r] = Ot
        # queues 1-3 hand descgen to background Q7 workers (the instruction
        # retires in ~80ns); round-robin so three rounds generate concurrently
        nc.gpsimd.dma_gather(
            Rt[:, 0:S, :],
            tab_d[:],
            idxs_t[:, SLOT_OFF[r] : SLOT_OFF[r + 1]],
            P * S,
            P * S,
            3 * F,
            queue_num=1 + (r % 3),
        )
        if r > 0:
            _finish(r - 1)
        # O[g] = A[g-1] + B[g]; r=0's leading bnd slot makes it one op, else
        # g=0's A comes from round r-1's last data row
        if r == 0:
            nc.vector.tensor_add(
                Ot[:, 0:G, :],
                Rt[:, 0:G, 0:F],
                Rt[:, 1 : G + 1, F : 2 * F],
            )
        else:
            if G > 1:
                nc.vector.tensor_add(
                    Ot[:, 1:G, :],
                    Rt[:, 0 : G - 1, 0:F],
                    Rt[:, 1:G, F : 2 * F],
                )
            pb, pG = BASE[r - 1], G_LIST[r - 1]
            nc.vector.tensor_add(
                Ot[:, 0:1, :],
                R[r - 1][:, pb + pG - 1 : pb + pG, 0:F],
                Rt[:, 0:1, F : 2 * F],
            )
        # C adds: O[g] += C[g+1]; last round's trailing bnd slot: one op
        if r == NR - 1:
            nc.vector.tensor_add(
                Ot[:, 0:G, :],
                Ot[:, 0:G, :],
                Rt[:, b + 1 : b + G + 1, 2 * F : 3 * F],
            )
        elif G > 1:
            nc.vector.tensor_add(
                Ot[:, 0 : G - 1, :],
                Ot[:, 0 : G - 1, :],
                Rt[:, b + 1 : b + G, 2 * F : 3 * F],
            )
    # last round: C was merged, store directly
    r, G = NR - 1, G_LIST[NR - 1]
    nc.sync.dma_start(out_d[:, G_OFF[r] : G_OFF[r + 1], :], O[r][:, 0:G, :])


def _build_nc():
    if "nc" in _nc_cache:
        return _nc_cache["nc"]
    nc = bacc.Bacc(
        "TRN2",
        target_bir_lowering=False,
        debug=False,
        enable_asserts=False,
        num_devices=N_CORES,
        dynamic_dma_scratch_size=DMA_SCRATCH,
        num_swdge_queues=4,
    )
    tab_d = nc.dram_tensor("tab", [VP, 3 * F], DT, kind="ExternalInput").ap()
    idxs_d = nc.dram_tensor(
        "idxs", [P, SW_TOT], mybir.dt.int16, kind="ExternalInput"
    ).ap()
    out_d = nc.dram_tensor("out", [P, NT, F], DTO, kind="ExternalOutput").ap()
    with tile.TileContext(nc) as tc:
        _gather_kernel(tc, out_d, tab_d, idxs_d)
    nc.compile()
    _nc_cache["nc"] = nc
    return nc


def _wrap16(stream):
    # gather idx wrap: idx i read from partition i%16, slot i//16; x8 replicas
    n = stream.shape[-1]
    w = stream.reshape(*stream.shape[:-1], n // 16, 16)
    w = np.swapaxes(w, -1, -2)  # [..., 16, n//16]
    reps = [1] * (w.ndim - 2) + [8, 1]
    return np.tile(w, reps)  # [..., 128, n//16]


def _host_prep(token_ids, conv_w):
    # TAB[v] = [A|B|C]: TAB[v, k*F+f] = round(conv_w[f, v, k] / delta);
    # int16 accumulation is exact, host applies delta (~7e-3 rel err)
    w = np.asarray(conv_w)
    delta = float(np.abs(w).max()) / 127.0
    tab = np.empty((VP, K * F), dtype=np.int8)
    q = np.rint(w.transpose(1, 2, 0).reshape(V, K * F) * (1.0 / delta))
    tab[:V] = np.clip(q, -127, 127).astype(np.int8)
    tab[V] = 0

    tok = np.asarray(token_ids).astype(np.int16)  # [B, T], V=32000 fits int16
    strip = tok.reshape(B, P, NT)

    # fused streams: per round r, slot s of the gather lands at dst[p, s];
    # stream[s*128 + p] = token for that slot. Round 0 has a leading strip-
    # edge slot (tok[p*NT-1], zero row at p=0); the last round a trailing
    # one (tok[(p+1)*NT], zero row at p=127).
    idxs = np.empty((B, P, SW_TOT), dtype=np.int16)
    for r, G in enumerate(G_LIST):
        S = S_LIST[r]
        x = np.empty((B, S, P), dtype=np.int16)  # [b, s, p]
        d0 = 0
        if r == 0:
            x[:, 0, 0] = V
            x[:, 0, 1:] = strip[:, :-1, NT - 1]
            d0 = 1
        x[:, d0 : d0 + G, :] = strip[:, :, G_OFF[r] : G_OFF[r + 1]].transpose(
            0, 2, 1
        )
        if r == NR - 1:
            x[:, S - 1, P - 1] = V
            x[:, S - 1, : P - 1] = strip[:, 1:, 0]
        stream = x.reshape(B, S * P)
        idxs[:, :, SLOT_OFF[r] : SLOT_OFF[r + 1]] = _wrap16(stream)
    return tab, np.ascontiguousarray(idxs), delta


def prepare(token_ids, conv_w):
    tab, idxs, delta = _host_prep(token_ids, conv_w)
    in_maps = [{"tab": tab, "idxs": idxs[b]} for b in range(B)]

    def post(res):
        # [P, NT, F] with t = p*NT + j flattens directly to [T, F]
        out = np.stack(
            [
                res.results[b]["out"].astype(np.float32).reshape(T, F)
                for b in range(B)
            ],
            axis=0,
        )
        out *= np.float32(delta)
        return np.ascontiguousarray(out)

    return in_maps, post


def kernel(token_ids, conv_w):
    in_maps, post = prepare(token_ids, conv_w)
    nc = _build_nc()
    res = run_bass_kernel_spmd(nc, in_maps, core_ids=list(range(N_CORES)))
    return post(res)


# revision 26
# speedup vs baseline: 1.0171x; 1.0171x over previous
"""CharCNN embedding kernel for Trainium2 (8 NeuronCores, Bass/Tile).

Computes out[b,t,f] = sum_k conv_w[f, token_ids[b, t+k-pad], k] with zero
padding outside [0,T) — i.e. one_hot(token_ids) -> Conv1d(V->F, k=3, pad=1).

Strategy: data-parallel over batch (B=8 rows, one per core), weight table
replicated and int8-quantized (w_i8 = round(w/delta), global delta =
max|w|/127; the 3-tap int16 accumulation is exact and the host applies
delta, ~7e-3 rel err vs the 2e-2 gate). int8 halves gather bytes so the
DMA window shrinks; the remaining wall is the DVE accumulation (int8-input
adds run ~1/2 the fp16 rate) plus a fixed ~16.5us gpsimd custom-ucode
library load at kernel start.

Key device-side structure per core:
  - fused table TAB [V+1, 3F] int8, TAB[v] = [A|B|C] = conv_w[:, v, :].T
    flattened (A=tap0, B=tap1, C=tap2), zero row at V for edge padding.
  - strip layout: partition p owns positions t = p*NT + j (j = 0..NT-1), so
    the +-1 tap shifts are free-dim shifts inside a partition.
  - per round of G strip-positions one dma_gather of 128*S fused 1.5KB rows
    (dst[i%128, i//128] = TAB[idx[i]], idx streamed in gather wrap order).
    Gathers are issued on SWDGE queues 1-3: nonzero queues hand descriptor
    generation to background Q7 workers (the gpsimd instruction retires in
    ~80ns), so all rounds' descgen runs concurrently and off the critical
    path (on queue 0 it would serialize at ~8.5ns/row). Rounds stay at
    <=1024 descriptors -- larger single calls overflow the ring and wedge
    the device.
  - the idx tile is loaded in two slices (round 0's slice first) so the
    first gather's semaphore wait clears as early as possible.
  - DVE accumulates O = A_shift + B (int8+int8->int16) then O += C_shift
    (int16+int8) into per-round int16 O tiles; every round has its own
    exact-size R/O buffer (unique pool tags) so there are no tile-reuse
    anti-deps and the DVE chain never stalls.
  - strip-edge boundary rows are folded into the first/last round gathers
    as an extra leading/trailing slot; the last round is small to shorten
    the post-DVE tail (store + end barrier).
Output DRAM layout [P, NT, F] int16 reshapes to [T, F] on host (x delta).
"""

from contextlib import ExitStack

import numpy as np

import concourse.bacc as bacc
import concourse.bass as bass
import concourse.mybir as mybir
import concourse.tile as tile
from concourse._compat import with_exitstack
from concourse.bass_utils import run_bass_kernel_spmd

B = 8
T = 4096
F = 512
V = 32000
VP = V + 1  # +1 zero row
K = 3
P = 128
NT = T // P  # 32 positions per partition strip
G_LIST = (2, 6, 8, 8, 7, 1)
NR = len(G_LIST)
G_OFF = tuple(int(x) for x in np.cumsum((0,) + G_LIST))  # round start offsets
# gathered slots per round: data rows + leading bnd slot (r=0) + trailing (last)
S_LIST = tuple(
    G + (1 if r == 0 else 0) + (1 if r == NR - 1 else 0)
    for r, G in enumerate(G_LIST)
)
SMAX = max(S_LIST)
GMAX = max(G_LIST)
SLOT_OFF = tuple(int(x) for x in np.cumsum((0,) + tuple(8 * s for s in S_LIST)))
SW_TOT = SLOT_OFF[-1]  # total idx slots per partition
# R_all slot map: slot 0 = leading bnd, slots 1..32 = positions 0..31,
# slot 33 = trailing bnd. Round r's gather writes slots SCUM[r]:SCUM[r+1].
SCUM = tuple(int(x) for x in np.cumsum((0,) + S_LIST))
NSLOT = SCUM[-1]
N_CORES = 8
DMA_SCRATCH = 24576
DT = mybir.dt.int8
DTO = mybir.dt.int16

_nc_cache = {}


@with_exitstack
def _gather_kernel(ctx: ExitStack, tc: tile.TileContext, out_d, tab_d, idxs_d):
    nc = tc.nc

    idxp = ctx.enter_context(tc.tile_pool(name="idx", bufs=1))
    # one exact-size buffer per round (unique tags, bufs=1): no tile-reuse
    # anti-deps anywhere, so the descgen chain never stalls on DVE/stores
    rp = ctx.enter_context(tc.tile_pool(name="rp", bufs=1))
    op = ctx.enter_context(tc.tile_pool(name="op", bufs=1))

    # split idx load: round 0's small slice lands (and its sem fires) sooner,
    # so the first gather issues earlier; the rest follows in parallel
    idxs_t = idxp.tile([P, SW_TOT], mybir.dt.int16)
    nc.sync.dma_start(idxs_t[:, 0 : SLOT_OFF[1]], idxs_d[:, 0 : SLOT_OFF[1]])
    nc.sync.dma_start(idxs_t[:, SLOT_OFF[1] :], idxs_d[:, SLOT_OFF[1] :])

    # one contiguous slot buffer for all rounds: slot s holds position s-1
    # (slot 0 / slot NSLOT-1 are the strip-edge boundary rows), so every
    # round's A/C adds are single ops spanning the round boundary — no
    # separate boundary-row adds on the DVE
    R_all = rp.tile([P, NSLOT, 3 * F], DT)
    O = [None] * NR

    def _a_op(r):
        # O[g] = A[pos g-1] + B[pos g]: slots G_OFF[r]..+G / G_OFF[r]+1..+G
        G = G_LIST[r]
        j = G_OFF[r]
        nc.vector.tensor_add(
            O[r][:, 0:G, :],
            R_all[:, j : j + G, 0:F],
            R_all[:, j + 1 : j + 1 + G, F : 2 * F],
        )

    def _c_op_store(r):
        # O[g] += C[pos g+1]: slots G_OFF[r]+2..+G (the last one is round
        # r+1's first data slot, or the trailing bnd slot); then store
        G = G_LIST[r]
        j = G_OFF[r]
        nc.vector.tensor_add(
            O[r][:, 0:G, :],
            O[r][:, 0:G, :],
            R_all[:, j + 2 : j + 2 + G, 2 * F : 3 * F],
        )
        nc.sync.dma_start(out_d[:, G_OFF[r] : G_OFF[r + 1], :], O[r][:, 0:G, :])

    for r, G in enumerate(G_LIST):
        S = S_LIST[r]
        O[r] = op.tile([P, G, F], DTO, tag=f"O{r}", name=f"O{r}")
        # queues 1-3 hand descgen to background Q7 workers (the instruction
        # retires in ~80ns); round-robin so three rounds generate concurrently
        nc.gpsimd.dma_gather(
            R_all[:, SCUM[r] : SCUM[r + 1], :],
            tab_d[:],
            idxs_t[:, SLOT_OFF[r] : SLOT_OFF[r + 1]],
            P * S,
            P * S,
            3 * F,
            queue_num=1 + (r % 3),
        )
        _a_op(r)
        if r > 0:
            _c_op_store(r - 1)
    _c_op_store(NR - 1)


def _build_nc():
    if "nc" in _nc_cache:
        return _nc_cache["nc"]
    nc = bacc.Bacc(
        "TRN2",
        target_bir_lowering=False,
        debug=False,
        enable_asserts=False,
        num_devices=N_CORES,
        dynamic_dma_scratch_size=DMA_SCRATCH,
        num_swdge_queues=4,
    )
    tab_d = nc.dram_tensor("tab", [VP, 3 * F], DT, kind="ExternalInput").ap()
    idxs_d = nc.dram_tensor(
        "idxs", [P, SW_TOT], mybir.dt.int16, kind="ExternalInput"
    ).ap()
    out_d = nc.dram_tensor("out", [P, NT, F], DTO, kind="ExternalOutput").ap()
    with tile.TileContext(nc) as tc:
        _gather_kernel(tc, out_d, tab_d, idxs_d)
    nc.compile()
    _nc_cache["nc"] = nc
    return nc


def _wrap16(stream):
    # gather idx wrap: idx i read from partition i%16, slot i//16; x8 replicas
    n = stream.shape[-1]
    w = stream.reshape(*stream.shape[:-1], n // 16, 16)
    w = np.swapaxes(w, -1, -2)  # [..., 16, n//16]
    reps = [1] * (w.ndim - 2) + [8, 1]
    return np.tile(w, reps)  # [..., 128, n//16]


def _host_prep(token_ids, conv_w):
    # TAB[v] = [A|B|C]: TAB[v, k*F+f] = round(conv_w[f, v, k] / delta);
    # int16 accumulation is exact, host applies delta (~7e-3 rel err)
    w = np.asarray(conv_w)
    delta = float(np.abs(w).max()) / 127.0
    tab = np.empty((VP, K * F), dtype=np.int8)
    q = np.rint(w.transpose(1, 2, 0).reshape(V, K * F) * (1.0 / delta))
    tab[:V] = np.clip(q, -127, 127).astype(np.int8)
    tab[V] = 0

    tok = np.asarray(token_ids).astype(np.int16)  # [B, T], V=32000 fits int16
    strip = tok.reshape(B, P, NT)

    # fused streams: per round r, slot s of the gather lands at dst[p, s];
    # stream[s*128 + p] = token for that slot. Round 0 has a leading strip-
    # edge slot (tok[p*NT-1], zero row at p=0); the last round a trailing
    # one (tok[(p+1)*NT], zero row at p=127).
    idxs = np.empty((B, P, SW_TOT), dtype=np.int16)
    for r, G in enumerate(G_LIST):
        S = S_LIST[r]
        x = np.empty((B, S, P), dtype=np.int16)  # [b, s, p]
        d0 = 0
        if r == 0:
            x[:, 0, 0] = V
            x[:, 0, 1:] = strip[:, :-1, NT - 1]
            d0 = 1
        x[:, d0 : d0 + G, :] = strip[:, :, G_OFF[r] : G_OFF[r + 1]].transpose(
            0, 2, 1
        )
        if r == NR - 1:
            x[:, S - 1, P - 1] = V
            x[:, S - 1, : P - 1] = strip[:, 1:, 0]
        stream = x.reshape(B, S * P)
        idxs[:, :, SLOT_OFF[r] : SLOT_OFF[r + 1]] = _wrap16(stream)
    return tab, np.ascontiguousarray(idxs), delta


def prepare(token_ids, conv_w):
    tab, idxs, delta = _host_prep(token_ids, conv_w)
    in_maps = [{"tab": tab, "idxs": idxs[b]} for b in range(B)]

    def post(res):
        # [P, NT, F] with t = p*NT + j flattens directly to [T, F]
        out = np.stack(
            [
                res.results[b]["out"].astype(np.float32).reshape(T, F)
                for b in range(B)
            ],
            axis=0,
        )
        out *= np.float32(delta)
        return np.ascontiguousarray(out)

    return in_maps, post


def kernel(token_ids, conv_w):
    in_maps, post = prepare(token_ids, conv_w)
    nc = _build_nc()
    res = run_bass_kernel_spmd(nc, in_maps, core_ids=list(range(N_CORES)))
    return post(res)


# revision 27
# speedup vs baseline: 1.0376x; 1.0201x over previous
"""CharCNN embedding kernel for Trainium2 (8 NeuronCores, Bass/Tile).

Computes out[b,t,f] = sum_k conv_w[f, token_ids[b, t+k-pad], k] with zero
padding outside [0,T) — i.e. one_hot(token_ids) -> Conv1d(V->F, k=3, pad=1).

Strategy: data-parallel over batch (B=8 rows, one per core), weight table
replicated and int8-quantized (w_i8 = round(w/delta), global delta =
max|w|/127; the 3-tap int16 accumulation is exact and the host applies
delta, ~7e-3 rel err vs the 2e-2 gate). int8 halves gather bytes so the
DMA window shrinks; the remaining wall is the DVE accumulation (int8-input
adds run ~1/2 the fp16 rate) plus a fixed ~16.5us gpsimd custom-ucode
library load at kernel start.

Key device-side structure per core:
  - fused table TAB [V+1, 3F] int8, TAB[v] = [A|B|C] = conv_w[:, v, :].T
    flattened (A=tap0, B=tap1, C=tap2), zero row at V for edge padding.
  - strip layout: partition p owns positions t = p*NT + j (j = 0..NT-1), so
    the +-1 tap shifts are free-dim shifts inside a partition.
  - per round of G strip-positions one dma_gather of 128*S fused 1.5KB rows
    (dst[i%128, i//128] = TAB[idx[i]], idx streamed in gather wrap order).
    Gathers are issued on SWDGE queues 1-3: nonzero queues hand descriptor
    generation to background Q7 workers (the gpsimd instruction retires in
    ~80ns), so all rounds' descgen runs concurrently and off the critical
    path (on queue 0 it would serialize at ~8.5ns/row). Rounds stay at
    <=1024 descriptors -- larger single calls overflow the ring and wedge
    the device.
  - the idx tile is loaded in two slices (round 0's slice first) so the
    first gather's semaphore wait clears as early as possible.
  - DVE accumulates O = A_shift + B (int8+int8->int16) then O += C_shift
    (int16+int8) into per-round int16 O tiles; every round has its own
    exact-size R/O buffer (unique pool tags) so there are no tile-reuse
    anti-deps and the DVE chain never stalls.
  - strip-edge boundary rows are folded into the first/last round gathers
    as an extra leading/trailing slot; the last round is small to shorten
    the post-DVE tail (store + end barrier).
Output DRAM layout [P, NT, F] int16 reshapes to [T, F] on host (x delta).
"""

from contextlib import ExitStack

import numpy as np

import concourse.bacc as bacc
import concourse.bass as bass
import concourse.mybir as mybir
import concourse.tile as tile
from concourse._compat import with_exitstack
from concourse.bass_utils import run_bass_kernel_spmd

B = 8
T = 4096
F = 512
V = 32000
VP = V + 1  # +1 zero row
K = 3
P = 128
NT = T // P  # 32 positions per partition strip
G_LIST = (2, 3, 5, 7, 7, 7, 1)
NR = len(G_LIST)
G_OFF = tuple(int(x) for x in np.cumsum((0,) + G_LIST))  # round start offsets
# gathered slots per round: data rows + leading bnd slot (r=0) + trailing (last)
S_LIST = tuple(
    G + (1 if r == 0 else 0) + (1 if r == NR - 1 else 0)
    for r, G in enumerate(G_LIST)
)
SMAX = max(S_LIST)
GMAX = max(G_LIST)
SLOT_OFF = tuple(int(x) for x in np.cumsum((0,) + tuple(8 * s for s in S_LIST)))
SW_TOT = SLOT_OFF[-1]  # total idx slots per partition
# R_all slot map: slot 0 = leading bnd, slots 1..32 = positions 0..31,
# slot 33 = trailing bnd. Round r's gather writes slots SCUM[r]:SCUM[r+1].
SCUM = tuple(int(x) for x in np.cumsum((0,) + S_LIST))
NSLOT = SCUM[-1]
N_CORES = 8
DMA_SCRATCH = 24576
DT = mybir.dt.int8
DTO = mybir.dt.int16

_nc_cache = {}


@with_exitstack
def _gather_kernel(ctx: ExitStack, tc: tile.TileContext, out_d, tab_d, idxs_d):
    nc = tc.nc

    idxp = ctx.enter_context(tc.tile_pool(name="idx", bufs=1))
    # one exact-size buffer per round (unique tags, bufs=1): no tile-reuse
    # anti-deps anywhere, so the descgen chain never stalls on DVE/stores
    rp = ctx.enter_context(tc.tile_pool(name="rp", bufs=1))
    op = ctx.enter_context(tc.tile_pool(name="op", bufs=1))

    # split idx load: round 0's small slice lands (and its sem fires) sooner,
    # so the first gather issues earlier; the rest follows in parallel
    idxs_t = idxp.tile([P, SW_TOT], mybir.dt.int16)
    nc.sync.dma_start(idxs_t[:, 0 : SLOT_OFF[1]], idxs_d[:, 0 : SLOT_OFF[1]])
    nc.sync.dma_start(idxs_t[:, SLOT_OFF[1] :], idxs_d[:, SLOT_OFF[1] :])

    # one contiguous slot buffer for all rounds: slot s holds position s-1
    # (slot 0 / slot NSLOT-1 are the strip-edge boundary rows), so every
    # round's A/C adds are single ops spanning the round boundary — no
    # separate boundary-row adds on the DVE
    R_all = rp.tile([P, NSLOT, 3 * F], DT)
    O = [None] * NR

    def _a_op(r):
        # O[g] = A[pos g-1] + B[pos g]: slots G_OFF[r]..+G / G_OFF[r]+1..+G
        G = G_LIST[r]
        j = G_OFF[r]
        nc.vector.tensor_add(
            O[r][:, 0:G, :],
            R_all[:, j : j + G, 0:F],
            R_all[:, j + 1 : j + 1 + G, F : 2 * F],
        )

    def _c_op_store(r):
        # O[g] += C[pos g+1]: slots G_OFF[r]+2..+G (the last one is round
        # r+1's first data slot, or the trailing bnd slot); then store
        G = G_LIST[r]
        j = G_OFF[r]
        nc.vector.tensor_add(
            O[r][:, 0:G, :],
            O[r][:, 0:G, :],
            R_all[:, j + 2 : j + 2 + G, 2 * F : 3 * F],
        )
        nc.sync.dma_start(out_d[:, G_OFF[r] : G_OFF[r + 1], :], O[r][:, 0:G, :])

    for r, G in enumerate(G_LIST):
        S = S_LIST[r]
        O[r] = op.tile([P, G, F], DTO, tag=f"O{r}", name=f"O{r}")
        # queues 1-3 hand descgen to background Q7 workers (the instruction
        # retires in ~80ns); round-robin so three rounds generate concurrently
        nc.gpsimd.dma_gather(
            R_all[:, SCUM[r] : SCUM[r + 1], :],
            tab_d[:],
            idxs_t[:, SLOT_OFF[r] : SLOT_OFF[r + 1]],
            P * S,
            P * S,
            3 * F,
            queue_num=1 + (r % 3),
        )
        _a_op(r)
        if r > 0:
            _c_op_store(r - 1)
    _c_op_store(NR - 1)


def _build_nc():
    if "nc" in _nc_cache:
        return _nc_cache["nc"]
    nc = bacc.Bacc(
        "TRN2",
        target_bir_lowering=False,
        debug=False,
        enable_asserts=False,
        num_devices=N_CORES,
        dynamic_dma_scratch_size=DMA_SCRATCH,
        num_swdge_queues=4,
    )
    tab_d = nc.dram_tensor("tab", [VP, 3 * F], DT, kind="ExternalInput").ap()
    idxs_d = nc.dram_tensor(
        "idxs", [P, SW_TOT], mybir.dt.int16, kind="ExternalInput"
    ).ap()
    out_d = nc.dram_tensor("out", [P, NT, F], DTO, kind="ExternalOutput").ap()
    with tile.TileContext(nc) as tc:
        _gather_kernel(tc, out_d, tab_d, idxs_d)
    nc.compile()
    _nc_cache["nc"] = nc
    return nc


def _wrap16(stream):
    # gather idx wrap: idx i read from partition i%16, slot i//16; x8 replicas
    n = stream.shape[-1]
    w = stream.reshape(*stream.shape[:-1], n // 16, 16)
    w = np.swapaxes(w, -1, -2)  # [..., 16, n//16]
    reps = [1] * (w.ndim - 2) + [8, 1]
    return np.tile(w, reps)  # [..., 128, n//16]


def _host_prep(token_ids, conv_w):
    # TAB[v] = [A|B|C]: TAB[v, k*F+f] = round(conv_w[f, v, k] / delta);
    # int16 accumulation is exact, host applies delta (~7e-3 rel err)
    w = np.asarray(conv_w)
    delta = float(np.abs(w).max()) / 127.0
    tab = np.empty((VP, K * F), dtype=np.int8)
    q = np.rint(w.transpose(1, 2, 0).reshape(V, K * F) * (1.0 / delta))
    tab[:V] = np.clip(q, -127, 127).astype(np.int8)
    tab[V] = 0

    tok = np.asarray(token_ids).astype(np.int16)  # [B, T], V=32000 fits int16
    strip = tok.reshape(B, P, NT)

    # fused streams: per round r, slot s of the gather lands at dst[p, s];
    # stream[s*128 + p] = token for that slot. Round 0 has a leading strip-
    # edge slot (tok[p*NT-1], zero row at p=0); the last round a trailing
    # one (tok[(p+1)*NT], zero row at p=127).
    idxs = np.empty((B, P, SW_TOT), dtype=np.int16)
    for r, G in enumerate(G_LIST):
        S = S_LIST[r]
        x = np.empty((B, S, P), dtype=np.int16)  # [b, s, p]
        d0 = 0
        if r == 0:
            x[:, 0, 0] = V
            x[:, 0, 1:] = strip[:, :-1, NT - 1]
            d0 = 1
        x[:, d0 : d0 + G, :] = strip[:, :, G_OFF[r] : G_OFF[r + 1]].transpose(
            0, 2, 1
        )
        if r == NR - 1:
            x[:, S - 1, P - 1] = V
            x[:, S - 1, : P - 1] = strip[:, 1:, 0]
        stream = x.reshape(B, S * P)
        idxs[:, :, SLOT_OFF[r] : SLOT_OFF[r + 1]] = _wrap16(stream)
    return tab, np.ascontiguousarray(idxs), delta


def prepare(token_ids, conv_w):
    tab, idxs, delta = _host_prep(token_ids, conv_w)
    in_maps = [{"tab": tab, "idxs": idxs[b]} for b in range(B)]

    def post(res):
        # [P, NT, F] with t = p*NT + j flattens directly to [T, F]
        out = np.stack(
            [
                res.results[b]["out"].astype(np.float32).reshape(T, F)
                for b in range(B)
            ],
            axis=0,
        )
        out *= np.float32(delta)
        return np.ascontiguousarray(out)

    return in_maps, post


def kernel(token_ids, conv_w):
    in_maps, post = prepare(token_ids, conv_w)
    nc = _build_nc()
    res = run_bass_kernel_spmd(nc, in_maps, core_ids=list(range(N_CORES)))
    return post(res)
